# revision 30
# baseline (speedup 1.0000x reference)
"""CAM (channel attention) module kernel for Trainium2, data-parallel over batch.

Computes, per sample:
    v = x.reshape(C, N)                  # N = H*W
    energy = v @ v.T                     # [C, C]
    att = softmax(rowmax(energy) - energy, axis=-1)
    out = gamma * (att @ v) + x

Distribution: batch B=32 split over 8 NeuronCores (4 samples/core), gamma
replicated.  Per core everything is computed on-chip.  Key optimizations over
the straightforward version:
  - v loaded once to SBUF (doubles as x for the residual add); v^T built with
    PE transpose-mode matmuls into a 4-D [128, KB, CB, 128] chunk-major tile
  - energy exploits symmetry: stripes s0=(0,0..3), s1=(1,1..3), s2=(2,2..3),
    s3=(3,2..3) are computed with fp32r matmuls (11 of 16 blocks, all with
    >=256 moving rows so fp32r runs at 1 cycle/row); the 5 remaining lower
    blocks are PE-transposed from upper blocks into the same PSUM banks as
    late members of each bank's accumulation group (start=False, last one
    carries stop)
  - softmax via softmax(rowmax - e) == exp(rowmin - e)/rowsum: row-min on
    DVE, exp (+ fused row-sum) on ACT; all row-mins are emitted before any
    reciprocal so the per-stripe chains pipeline instead of serializing
  - the unnormalized attention is transposed with 16 PE transposes (emitted
    cb-major so only the last batch waits on the last exp) and cast to
    fp8e4m3 in the PSUM->SBUF copy; v is cast to fp8 on the (otherwise idle)
    GPSIMD engine; the second matmul then runs in fp8 DoubleRow mode (2
    k-tiles of 128 per instruction, 2x MAC throughput); row normalization
    (1/Z) and gamma fold into one per-partition scalar in the epilogue
  - epilogue fuses (psum * (gamma/Z)) + x in one DVE pass, writing bf16,
    which halves the output DMA traffic (host upcasts to fp32)
  - deep cross-sample software pipelining: each sample's mm2+epilogue loop
    hosts the NEXT sample's v-transposes (paced to the DMA arrival of the
    three load ranges), energy accumulation (2 slots behind each transpose),
    fp8 cast and softmax/attention-transpose head, so PE/DVE/ACT/Pool/DMA
    all stay busy across the whole period instead of phase-bunching.
    Engine queues matter as much as totals on hardware: the transpose-train
    copies all go to ACT (routing any through the epilogue-loaded DVE queue
    amplifies its latency), est/pt8 staging is on ACT, row-min/recip/
    epilogue on DVE, fp8 casts on GPSIMD (which cannot touch PSUM).
"""

import sys

sys.path.insert(0, "/opt/trn_rl_repo")

from contextlib import ExitStack

import numpy as np

import concourse.bacc as bacc
import concourse.bass as bass
import concourse.mybir as mybir
import concourse.tile as tile
from concourse import masks
from concourse.bass_utils import run_bass_kernel_spmd

B, C, H, W = 32, 512, 48, 48
N = H * W  # 2304
NCORES = 8
SPC = B // NCORES  # samples per core
P = 128
CB = C // P  # 4 channel blocks
KB = N // P  # 18 spatial chunks of 128

# energy stripe runs: stripe ib computes blocks (ib, jb) for jb in
# [EST[ib], EST[ib]+EW[ib]); every run is >=2 blocks so fp32r streams >=256
# rows per matmul.  (3,2) double-computes pair {2,3} to avoid a 128-wide run.
EST = [0, 1, 2, 2]
EW = [4, 3, 2, 2]

FP32 = mybir.dt.float32
FP32R = mybir.dt.float32r
FP8 = mybir.dt.float8e4
BF16 = mybir.dt.bfloat16
AX = mybir.AxisListType.X
OP = mybir.AluOpType
AF = mybir.ActivationFunctionType
DR = mybir.MatmulPerfMode.DoubleRow


def _emit(tc, ctx, x, gamma, out, reps=1):
    nc = tc.nc

    const_pool = ctx.enter_context(tc.tile_pool(name="const", bufs=1))
    ident_f32 = const_pool.tile([P, P], FP32)
    masks.make_identity(nc, ident_f32[:])
    ident = const_pool.tile([P, P], FP32R)
    nc.scalar.copy(ident[:], ident_f32[:])
    gamma_sb = const_pool.tile([P, 1], FP32)
    nc.sync.dma_start(gamma_sb[:], bass.AP(gamma.tensor, 0, [[0, P], [1, 1]]))

    v_pool = ctx.enter_context(tc.tile_pool(name="v", bufs=3))
    vt_pool = ctx.enter_context(tc.tile_pool(name="vt", bufs=1))
    v8_pool = ctx.enter_context(tc.tile_pool(name="v8", bufs=2))
    p_pool = ctx.enter_context(tc.tile_pool(name="p", bufs=1))
    pt_pool = ctx.enter_context(tc.tile_pool(name="pt", bufs=2))
    e_pool = ctx.enter_context(tc.tile_pool(name="est", bufs=1))
    o_pool = ctx.enter_context(tc.tile_pool(name="o", bufs=3))
    vec_pool = ctx.enter_context(tc.tile_pool(name="vec", bufs=4))
    s_pool = ctx.enter_context(tc.tile_pool(name="s", bufs=2))
    # PSUM budget is exactly 8 banks: energy/attn-T share a 4-bank slot
    # (their lifetimes are disjoint), 2 rotating transpose banks, 2 output
    # banks.
    ps_e = ctx.enter_context(tc.tile_pool(name="ps_e", bufs=1, space="PSUM"))
    ps_t = ctx.enter_context(tc.tile_pool(name="ps_t", bufs=2, space="PSUM"))
    ps_o = ctx.enter_context(tc.tile_pool(name="ps_o", bufs=2, space="PSUM"))

    nsamp = reps * SPC
    v_t = {}
    vt_t = {}
    v8_t = {}
    e_t = {}
    p_t = {}
    sa_t = {}
    pt8_t = {}
    est_t = {}
    ptps_t = {}

    def load_v(i):
        # split per c-block into 3 column ranges so the first transposes can
        # start before the whole sample has landed
        s = i % SPC
        v = v_pool.tile([P, CB * N], FP32R, tag="v", name=f"v{i}")
        for a, b in ((0, 768), (768, 1536), (1536, N)):
            for cb in range(CB):
                nc.sync.dma_start(
                    v[:, cb * N + a : cb * N + b],
                    x[s, cb * P : (cb + 1) * P, a:b].bitcast(FP32R),
                )
        v_t[i] = v

    def a_chunk(i, k):
        # transpose one 128-wide spatial chunk of v into the 3-D block-major
        # vt tile; all PSUM->SBUF copies go to ACT: DVE's queue (epilogue)
        # is long, and routing the train through it amplifies its latency
        if k == 0:
            vt_t[i] = vt_pool.tile([P, KB, CB, P], FP32R, tag="vt", name=f"vt{i}")
        v, vt = v_t[i], vt_t[i]
        tps = ps_t.tile([P, 512], FP32R, tag="tps")
        for cb in range(CB):
            nc.tensor.matmul(
                tps[:, cb * P : (cb + 1) * P],
                v[:, cb * N + k * P : cb * N + (k + 1) * P],
                ident[:],
                is_transpose=True,
                start=(cb == 0),
                stop=(cb == CB - 1),
            )
        nc.scalar.copy(vt[:, k, :, :], tps[:])

    def cast_v8(i):
        # v -> fp8 for the DoubleRow matmul (gpsimd; SBUF->SBUF).  Emitted a
        # sample ahead so the Pool engine has a full sample of slack.
        v8 = v8_pool.tile([P, CB, N], FP8, tag="v8", name=f"v8_{i}")
        for cb in range(CB):
            nc.gpsimd.tensor_copy(
                v8[:, cb : cb + 1, :], v_t[i][:, cb * N : (cb + 1) * N]
            )
        v8_t[i] = v8

    def e_chunk(i, k):
        # accumulate chunk k of all 4 energy stripes for sample i.  The
        # stripe-ib group opens at k==0; only stripe 0 closes at k==17 (the
        # others close via their transposed-in lower blocks)
        if k == 0:
            e_t[i] = ps_e.tile([P, CB * 512], FP32, tag="eb", name=f"e{i}")
        energy, vt = e_t[i], vt_t[i]
        for ib in range(CB):
            j0, w = EST[ib], EW[ib]
            nc.tensor.matmul(
                energy[:, ib * 512 + j0 * P : ib * 512 + (j0 + w) * P],
                vt[:, k, ib, :],
                vt[:, k, j0 : j0 + w, :],
                start=(k == 0),
                stop=(k == KB - 1 and ib == 0),
            )

    sm_t = {}

    def sm_min(i, ib):
        # row-min of stripe ib (DVE); emitted for all stripes before any
        # reciprocal so the per-stripe chains don't serialize the DVE queue
        mn = vec_pool.tile([P, 1], FP32, tag="mn", name=f"mn{i}_{ib}")
        nc.vector.tensor_reduce(
            mn[:], e_t[i][:, ib * 512 : (ib + 1) * 512], axis=AX, op=OP.min
        )
        sm_t[(i, ib)] = mn

    def sm_exp(i, ib):
        # softmax(rowmax - e) == exp(rowmin - e) / rowsum; exp + fused
        # row-sum on ACT
        z = vec_pool.tile([P, 1], FP32, tag="z", name=f"z{i}_{ib}")
        nc.scalar.activation(
            p_t[i][:, ib * 512 : (ib + 1) * 512],
            e_t[i][:, ib * 512 : (ib + 1) * 512],
            AF.Exp,
            bias=sm_t[(i, ib)][:],
            scale=-1.0,
            accum_out=z[:],
        )
        sm_t[(i, ib)] = z

    def sm_fin(i, ib):
        z = sm_t.pop((i, ib))
        r = vec_pool.tile([P, 1], FP32, tag="r", name=f"r{i}_{ib}")
        nc.vector.reciprocal(r[:], z[:])
        nc.vector.tensor_tensor(
            sa_t[i][:, ib : ib + 1], r[:], gamma_sb[:], op=OP.mult
        )

    def softmax(i, ib):
        sm_min(i, ib)
        sm_exp(i, ib)
        sm_fin(i, ib)

    def lowt(i, dib, djb, src, last):
        # transpose a staged upper block into a lower slot of the still-
        # open stripe-dib group (start=False; `last` carries the stop)
        nc.tensor.matmul(
            e_t[i][:, dib * 512 + djb * P : dib * 512 + (djb + 1) * P].bitcast(
                FP32R
            ),
            src,
            ident[:],
            is_transpose=True,
            start=False,
            stop=last,
        )

    def head_a(i):
        # finish the energy accumulation; stripe 0 is then closed: run its
        # softmax and stage its off-diagonal blocks for the lower transposes
        e_chunk(i, KB - 2)
        e_chunk(i, KB - 1)
        p_t[i] = p_pool.tile([P, CB * 512], FP32R, tag="p", name=f"p{i}")
        sa_t[i] = s_pool.tile([P, CB], FP32, tag="s", name=f"sa{i}")
        softmax(i, 0)
        estA = est_t[i] = e_pool.tile([P, 3, P], FP32R, tag="estA", name=f"estA{i}")
        for j, src_jb in enumerate([1, 2, 3]):
            nc.scalar.copy(
                estA[:, j : j + 1, :],
                e_t[i][:, src_jb * P : (src_jb + 1) * P],
            )

    def head_b(i):
        # T(1,0) closes stripe 1; only then can stripe 1's blocks be staged
        # (the race detector forbids reading a bank mid-accumulation-group).
        # All row-mins are emitted before any reciprocal so the per-stripe
        # mn->exp->recip chains pipeline instead of serializing the DVE queue.
        estA = est_t.pop(i)
        lowt(i, 1, 0, estA[:, 0:1, :], True)
        sm_min(i, 1)
        estB = e_pool.tile([P, 2, P], FP32R, tag="estB")
        for j, src_jb in enumerate([2, 3]):
            nc.scalar.copy(
                estB[:, j : j + 1, :],
                e_t[i][:, 512 + src_jb * P : 512 + (src_jb + 1) * P],
            )
        sm_exp(i, 1)
        lowt(i, 2, 0, estA[:, 1:2, :], False)
        lowt(i, 3, 0, estA[:, 2:3, :], False)
        lowt(i, 2, 1, estB[:, 0:1, :], True)
        lowt(i, 3, 1, estB[:, 1:2, :], True)
        sm_min(i, 2)
        sm_min(i, 3)
        sm_exp(i, 2)
        sm_exp(i, 3)
        sm_fin(i, 1)
        sm_fin(i, 2)
        sm_fin(i, 3)

    def head_c(i, cb):
        # transpose the cb-sourced blocks of the unnormalized attention into
        # all 4 banks (cb-major: each batch runs right after exp cb, so only
        # the last batch waits for the final exp).  Reuses the energy banks
        # (tag "eb"); the fp8 cast-copies run on ACT (DVE owns the epilogue)
        p_sb = p_t[i]
        if cb == 0:
            del e_t[i]
            pt8_t[i] = pt_pool.tile([P, CB, 512], FP8, tag="pt", name=f"pt8_{i}")
            ptps_t[i] = ps_e.tile([P, CB * 512], FP32R, tag="eb", name=f"ptps{i}")
        pt_ps, pt8 = ptps_t[i], pt8_t[i]
        for db in range(CB):
            nc.tensor.matmul(
                pt_ps[:, db * 512 + cb * P : db * 512 + (cb + 1) * P],
                p_sb[:, cb * 512 + db * P : cb * 512 + (db + 1) * P],
                ident[:],
                is_transpose=True,
                start=(cb == 0),
                stop=(cb == CB - 1),
            )
        if cb == CB - 1:
            for db in range(CB):
                nc.scalar.copy(
                    pt8[:, db : db + 1, :], pt_ps[:, db * 512 : (db + 1) * 512]
                )
            del ptps_t[i]

    # per-slot schedule for next-sample work inside the 20-slot mm2 loop:
    # vt transposes paced to the v-load DMA arrival (2 chunks/slot), energy
    # chunks 2 behind, then the head (softmax / attention transpose) spread
    # over the late slots so its DVE/ACT latency hides under mm2+epilogue
    A_SCHED = {3: (0, 1, 2), 4: (3, 4, 5), 5: (6, 7), 6: (8, 9), 7: (10, 11),
               8: (12, 13), 9: (14, 15), 10: (16, 17)}
    E_SCHED = {5: (0, 1), 6: (2, 3), 7: (4, 5), 8: (6, 7), 9: (8, 9),
               10: (10, 11), 11: (12, 13), 12: (14, 15)}

    def emit(i):
        # Deeply software-pipelined: when emit(i) starts, sample i's vt,
        # energy, softmax and attention transpose are ALL already emitted
        # (the "head" ran interleaved into sample i-1's matmul phase).  This
        # emit runs sample i's mm2 + epilogue with sample i+1's work
        # interleaved per A_SCHED/E_SCHED.
        s = i % SPC
        v, vt, v8, pt8 = v_t[i], vt_t[i], v8_t.pop(i), pt8_t.pop(i)
        s_all = sa_t[i]
        nxt = i + 1 < nsamp
        if nxt:
            load_v(i + 1)

        slot = 0
        for cb in range(CB):
            ot = o_pool.tile([P, N], BF16, tag="ot")
            n_off = 0
            for nch in (512, 512, 512, 512, 256):
                po = ps_o.tile([P, 512], FP32, tag="po")
                for pair in range(2):
                    nc.tensor.matmul(
                        po[:, :nch],
                        pt8[:, 2 * pair : 2 * pair + 2, cb * P : (cb + 1) * P],
                        v8[:, 2 * pair : 2 * pair + 2, n_off : n_off + nch],
                        start=(pair == 0),
                        stop=(pair == 1),
                        perf_mode=DR,
                    )
                nc.vector.scalar_tensor_tensor(
                    ot[:, n_off : n_off + nch],
                    po[:, :nch],
                    s_all[:, cb : cb + 1],
                    v[:, cb * N + n_off : cb * N + n_off + nch].bitcast(FP32),
                    op0=OP.mult,
                    op1=OP.add,
                )
                n_off += nch
                slot += 1
                if nxt:
                    for k in A_SCHED.get(slot, ()):
                        a_chunk(i + 1, k)
                    for k in E_SCHED.get(slot, ()):
                        e_chunk(i + 1, k)
                    if slot == 9:
                        cast_v8(i + 1)
                    elif slot == 12:
                        head_a(i + 1)
                    elif slot == 13:
                        head_b(i + 1)
                    elif 14 <= slot <= 17:
                        head_c(i + 1, slot - 14)  # cb-major batch
            nc.sync.dma_start(out[s, cb * P : (cb + 1) * P, :], ot[:])
        del v_t[i], vt_t[i], p_t[i], sa_t[i]

    load_v(0)
    for k in range(KB):
        a_chunk(0, k)
    for k in range(KB - 2):
        e_chunk(0, k)
    cast_v8(0)
    head_a(0)
    head_b(0)
    for cb_ in range(CB):
        head_c(0, cb_)
    for i in range(nsamp):
        emit(i)


_nc_cache = {}


def _build(reps=1):
    if reps in _nc_cache:
        return _nc_cache[reps]
    nc = bacc.Bacc("TRN2", target_bir_lowering=False, debug=False)
    x_d = nc.dram_tensor("x", [SPC, C, N], FP32, kind="ExternalInput")
    g_d = nc.dram_tensor("gamma", [1], FP32, kind="ExternalInput")
    o_d = nc.dram_tensor("out", [SPC, C, N], BF16, kind="ExternalOutput")
    with tile.TileContext(nc) as tc, ExitStack() as ctx:
        _emit(tc, ctx, x_d.ap(), g_d.ap(), o_d.ap(), reps=reps)
    nc.compile()
    _nc_cache[reps] = nc
    return nc


def _bench_fn(reps, x, gamma):
    """Build a jitted 8-core executor for the reps-times-repeated kernel with
    device-resident inputs.  Used by test.py for differential timing."""
    import jax
    from jax.experimental.shard_map import shard_map
    from jax.sharding import Mesh, NamedSharding, PartitionSpec

    from concourse import bass2jax

    bass2jax.install_neuronx_cc_hook()
    nc = _build(reps=reps)
    pid = nc.partition_id_tensor.name if nc.partition_id_tensor else None
    in_names, out_names, out_avals, zero_outs = [], [], [], []
    for alloc in nc.m.functions[0].allocations:
        if not isinstance(alloc, mybir.MemoryLocationSet):
            continue
        name = alloc.memorylocations[0].name
        if alloc.kind == "ExternalInput":
            if name != pid:
                in_names.append(name)
        elif alloc.kind == "ExternalOutput":
            out_names.append(name)
            shape = tuple(alloc.tensor_shape)
            dtype = mybir.dt.np(alloc.dtype)
            out_avals.append(jax.core.ShapedArray(shape, dtype))
            zero_outs.append(np.zeros(shape, dtype))
    all_in_names = list(in_names) + list(out_names)
    if pid:
        all_in_names.append(pid)

    def _body(*args):
        operands = list(args)
        if pid:
            operands.append(bass2jax.partition_id_tensor())
        return tuple(
            bass2jax._bass_exec_p.bind(
                *operands,
                out_avals=tuple(out_avals),
                in_names=tuple(all_in_names),
                out_names=tuple(out_names),
                lowering_input_output_aliases=(),
                sim_require_finite=True,
                sim_require_nnan=True,
                nc=nc,
            )
        )

    devices = jax.devices()[:NCORES]
    mesh = Mesh(np.asarray(devices), ("core",))
    specs = (PartitionSpec("core"),) * (len(in_names) + len(out_names))
    fn = jax.jit(
        shard_map(
            _body,
            mesh=mesh,
            in_specs=specs,
            out_specs=(PartitionSpec("core"),) * len(out_names),
            check_rep=False,
        ),
        keep_unused=True,
    )
    sh = NamedSharding(mesh, PartitionSpec("core"))
    ins = {
        "x": np.ascontiguousarray(x, dtype=np.float32).reshape(B, C, N),
        "gamma": np.tile(np.ascontiguousarray(gamma, dtype=np.float32), (NCORES,)),
    }
    args = [jax.device_put(ins[n], sh) for n in in_names]
    args += [
        jax.device_put(np.zeros((NCORES * z.shape[0], *z.shape[1:]), z.dtype), sh)
        for z in zero_outs
    ]
    return fn, args


def kernel(x: np.ndarray, gamma: np.ndarray, **run_kwargs) -> np.ndarray:
    assert x.shape == (B, C, H, W), x.shape
    nc = _build()
    xr = np.ascontiguousarray(x, dtype=np.float32).reshape(B, C, N)
    g = np.ascontiguousarray(gamma, dtype=np.float32)
    in_maps = [
        {"x": xr[g_idx * SPC : (g_idx + 1) * SPC], "gamma": g}
        for g_idx in range(NCORES)
    ]
    res = run_bass_kernel_spmd(nc, in_maps, core_ids=list(range(NCORES)), **run_kwargs)
    outs = [np.asarray(res.results[g_idx]["out"]) for g_idx in range(NCORES)]
    full = (
        np.concatenate(outs, axis=0)
        .astype(np.float32)
        .reshape(B, C, H, W)
    )
    if run_kwargs:
        kernel.last_results = res
    return full


# revision 31
# speedup vs baseline: 1.1096x; 1.1096x over previous
"""CAM (channel attention) module kernel for Trainium2, data-parallel over batch.

Computes, per sample:
    v = x.reshape(C, N)                  # N = H*W
    energy = v @ v.T                     # [C, C]
    att = softmax(rowmax(energy) - energy, axis=-1)
    out = gamma * (att @ v) + x

Distribution: batch B=32 split over 8 NeuronCores (4 samples/core), gamma
replicated.  Per core everything is computed on-chip.  Key optimizations over
the straightforward version:
  - v loaded once to SBUF (doubles as x for the residual add); v^T built with
    PE transpose-mode matmuls into a 4-D [128, KB, CB, 128] chunk-major tile
  - energy exploits symmetry: stripes s0=(0,0..3), s1=(1,1..3), s2=(2,2..3),
    s3=(3,2..3) are computed with fp32r matmuls (11 of 16 blocks, all with
    >=256 moving rows so fp32r runs at 1 cycle/row); the 5 remaining lower
    blocks are PE-transposed from upper blocks into the same PSUM banks as
    late members of each bank's accumulation group (start=False, last one
    carries stop)
  - softmax via softmax(rowmax - e) == exp(rowmin - e)/rowsum: row-min on
    DVE, exp (+ fused row-sum) on ACT; all row-mins are emitted before any
    reciprocal so the per-stripe chains pipeline instead of serializing
  - the unnormalized attention is transposed with 16 PE transposes (emitted
    cb-major so only the last batch waits on the last exp) and cast to
    fp8e4m3 in the PSUM->SBUF copy; v is cast to fp8 on the (otherwise idle)
    GPSIMD engine; the second matmul then runs in fp8 DoubleRow mode (2
    k-tiles of 128 per instruction, 2x MAC throughput); row normalization
    (1/Z) and gamma fold into one per-partition scalar in the epilogue
  - epilogue fuses (psum * (gamma/Z)) + x in one DVE pass, writing bf16,
    which halves the output DMA traffic (host upcasts to fp32)
  - deep cross-sample software pipelining: each sample's mm2+epilogue loop
    hosts the NEXT sample's v-transposes (paced to the DMA arrival of the
    three load ranges), energy accumulation (2 slots behind each transpose),
    fp8 cast and softmax/attention-transpose head, so PE/DVE/ACT/Pool/DMA
    all stay busy across the whole period instead of phase-bunching.
    Engine queues matter as much as totals on hardware: the transpose-train
    copies all go to ACT (routing any through the epilogue-loaded DVE queue
    amplifies its latency), est/pt8 staging is on ACT, row-min/recip/
    epilogue on DVE, fp8 casts on GPSIMD (which cannot touch PSUM).
"""

import sys

sys.path.insert(0, "/opt/trn_rl_repo")

from contextlib import ExitStack

import numpy as np

import concourse.bacc as bacc
import concourse.bass as bass
import concourse.mybir as mybir
import concourse.tile as tile
from concourse import masks
from concourse.bass_utils import run_bass_kernel_spmd

B, C, H, W = 32, 512, 48, 48
N = H * W  # 2304
NCORES = 8
SPC = B // NCORES  # samples per core
P = 128
CB = C // P  # 4 channel blocks
KB = N // P  # 18 spatial chunks of 128

# energy stripe runs: stripe ib computes blocks (ib, jb) for jb in
# [EST[ib], EST[ib]+EW[ib]); every run is >=2 blocks so fp32r streams >=256
# rows per matmul.  (3,2) double-computes pair {2,3} to avoid a 128-wide run.
EST = [0, 1, 2, 2]
EW = [4, 3, 2, 2]

FP32 = mybir.dt.float32
FP32R = mybir.dt.float32r
FP8 = mybir.dt.float8e4
BF16 = mybir.dt.bfloat16
AX = mybir.AxisListType.X
OP = mybir.AluOpType
AF = mybir.ActivationFunctionType
DR = mybir.MatmulPerfMode.DoubleRow


def _emit(tc, ctx, x, gamma, out, reps=1):
    nc = tc.nc

    const_pool = ctx.enter_context(tc.tile_pool(name="const", bufs=1))
    ident_f32 = const_pool.tile([P, P], FP32)
    masks.make_identity(nc, ident_f32[:])
    ident = const_pool.tile([P, P], FP32R)
    nc.scalar.copy(ident[:], ident_f32[:])
    gamma_sb = const_pool.tile([P, 1], FP32)
    nc.sync.dma_start(gamma_sb[:], bass.AP(gamma.tensor, 0, [[0, P], [1, 1]]))

    v_pool = ctx.enter_context(tc.tile_pool(name="v", bufs=3))
    vt_pool = ctx.enter_context(tc.tile_pool(name="vt", bufs=1))
    v8_pool = ctx.enter_context(tc.tile_pool(name="v8", bufs=2))
    p_pool = ctx.enter_context(tc.tile_pool(name="p", bufs=1))
    pt_pool = ctx.enter_context(tc.tile_pool(name="pt", bufs=2))
    e_pool = ctx.enter_context(tc.tile_pool(name="est", bufs=1))
    o_pool = ctx.enter_context(tc.tile_pool(name="o", bufs=4))
    vec_pool = ctx.enter_context(tc.tile_pool(name="vec", bufs=4))
    s_pool = ctx.enter_context(tc.tile_pool(name="s", bufs=2))
    # PSUM budget is exactly 8 banks: energy/attn-T share a 4-bank slot
    # (their lifetimes are disjoint), 2 rotating transpose banks, 2 output
    # banks.
    ps_e = ctx.enter_context(tc.tile_pool(name="ps_e", bufs=1, space="PSUM"))
    ps_t = ctx.enter_context(tc.tile_pool(name="ps_t", bufs=2, space="PSUM"))
    ps_o = ctx.enter_context(tc.tile_pool(name="ps_o", bufs=2, space="PSUM"))

    nsamp = reps * SPC
    v_t = {}
    vt_t = {}
    v8_t = {}
    e_t = {}
    p_t = {}
    sa_t = {}
    pt8_t = {}
    est_t = {}
    ptps_t = {}

    def load_v(i):
        # split per c-block into 3 column ranges so the first transposes can
        # start before the whole sample has landed
        s = i % SPC
        v = v_pool.tile([P, CB * N], FP32R, tag="v", name=f"v{i}")
        for a, b in ((0, 768), (768, 1536), (1536, N)):
            for cb in range(CB):
                nc.sync.dma_start(
                    v[:, cb * N + a : cb * N + b],
                    x[s, cb * P : (cb + 1) * P, a:b].bitcast(FP32R),
                )
        v_t[i] = v

    def a_chunk(i, k):
        # transpose one 128-wide spatial chunk of v into the 3-D block-major
        # vt tile; all PSUM->SBUF copies go to ACT: DVE's queue (epilogue)
        # is long, and routing the train through it amplifies its latency
        if k == 0:
            vt_t[i] = vt_pool.tile([P, KB, CB, P], FP32R, tag="vt", name=f"vt{i}")
        v, vt = v_t[i], vt_t[i]
        tps = ps_t.tile([P, 512], FP32R, tag="tps")
        for cb in range(CB):
            nc.tensor.matmul(
                tps[:, cb * P : (cb + 1) * P],
                v[:, cb * N + k * P : cb * N + (k + 1) * P],
                ident[:],
                is_transpose=True,
                start=(cb == 0),
                stop=(cb == CB - 1),
            )
        nc.scalar.copy(vt[:, k, :, :], tps[:])

    def cast_v8(i):
        # v -> fp8 for the DoubleRow matmul (gpsimd; SBUF->SBUF).  Emitted a
        # sample ahead so the Pool engine has a full sample of slack.
        v8 = v8_pool.tile([P, CB, N], FP8, tag="v8", name=f"v8_{i}")
        for cb in range(CB):
            nc.gpsimd.tensor_copy(
                v8[:, cb : cb + 1, :], v_t[i][:, cb * N : (cb + 1) * N]
            )
        v8_t[i] = v8

    def e_chunk(i, k):
        # accumulate chunk k of all 4 energy stripes for sample i.  The
        # stripe-ib group opens at k==0; only stripe 0 closes at k==17 (the
        # others close via their transposed-in lower blocks)
        if k == 0:
            e_t[i] = ps_e.tile([P, CB * 512], FP32, tag="eb", name=f"e{i}")
        energy, vt = e_t[i], vt_t[i]
        for ib in range(CB):
            j0, w = EST[ib], EW[ib]
            nc.tensor.matmul(
                energy[:, ib * 512 + j0 * P : ib * 512 + (j0 + w) * P],
                vt[:, k, ib, :],
                vt[:, k, j0 : j0 + w, :],
                start=(k == 0),
                stop=(k == KB - 1 and ib == 0),
            )

    sm_t = {}

    def sm_min(i, ib):
        # row-min of stripe ib (DVE); emitted for all stripes before any
        # reciprocal so the per-stripe chains don't serialize the DVE queue
        mn = vec_pool.tile([P, 1], FP32, tag="mn", name=f"mn{i}_{ib}")
        nc.vector.tensor_reduce(
            mn[:], e_t[i][:, ib * 512 : (ib + 1) * 512], axis=AX, op=OP.min
        )
        sm_t[(i, ib)] = mn

    def sm_exp(i, ib):
        # softmax(rowmax - e) == exp(rowmin - e) / rowsum; exp + fused
        # row-sum on ACT
        z = vec_pool.tile([P, 1], FP32, tag="z", name=f"z{i}_{ib}")
        nc.scalar.activation(
            p_t[i][:, ib * 512 : (ib + 1) * 512],
            e_t[i][:, ib * 512 : (ib + 1) * 512],
            AF.Exp,
            bias=sm_t[(i, ib)][:],
            scale=-1.0,
            accum_out=z[:],
        )
        sm_t[(i, ib)] = z

    def sm_fin(i, ib):
        z = sm_t.pop((i, ib))
        r = vec_pool.tile([P, 1], FP32, tag="r", name=f"r{i}_{ib}")
        nc.vector.reciprocal(r[:], z[:])
        nc.vector.tensor_tensor(
            sa_t[i][:, ib : ib + 1], r[:], gamma_sb[:], op=OP.mult
        )

    def softmax(i, ib):
        sm_min(i, ib)
        sm_exp(i, ib)
        sm_fin(i, ib)

    def lowt(i, dib, djb, src, last):
        # transpose a staged upper block into a lower slot of the still-
        # open stripe-dib group (start=False; `last` carries the stop)
        nc.tensor.matmul(
            e_t[i][:, dib * 512 + djb * P : dib * 512 + (djb + 1) * P].bitcast(
                FP32R
            ),
            src,
            ident[:],
            is_transpose=True,
            start=False,
            stop=last,
        )

    def head_a(i):
        # finish the energy accumulation; stripe 0 is then closed: run its
        # softmax and stage its off-diagonal blocks for the lower transposes
        e_chunk(i, KB - 2)
        e_chunk(i, KB - 1)
        p_t[i] = p_pool.tile([P, CB * 512], FP32R, tag="p", name=f"p{i}")
        sa_t[i] = s_pool.tile([P, CB], FP32, tag="s", name=f"sa{i}")
        softmax(i, 0)
        estA = est_t[i] = e_pool.tile([P, 3, P], FP32R, tag="estA", name=f"estA{i}")
        for j, src_jb in enumerate([1, 2, 3]):
            nc.scalar.copy(
                estA[:, j : j + 1, :],
                e_t[i][:, src_jb * P : (src_jb + 1) * P],
            )

    def head_b(i):
        # T(1,0) closes stripe 1; only then can stripe 1's blocks be staged
        # (the race detector forbids reading a bank mid-accumulation-group).
        # All row-mins are emitted before any reciprocal so the per-stripe
        # mn->exp->recip chains pipeline instead of serializing the DVE queue.
        estA = est_t.pop(i)
        lowt(i, 1, 0, estA[:, 0:1, :], True)
        sm_min(i, 1)
        estB = e_pool.tile([P, 2, P], FP32R, tag="estB")
        for j, src_jb in enumerate([2, 3]):
            nc.scalar.copy(
                estB[:, j : j + 1, :],
                e_t[i][:, 512 + src_jb * P : 512 + (src_jb + 1) * P],
            )
        sm_exp(i, 1)
        lowt(i, 2, 0, estA[:, 1:2, :], False)
        lowt(i, 3, 0, estA[:, 2:3, :], False)
        lowt(i, 2, 1, estB[:, 0:1, :], True)
        lowt(i, 3, 1, estB[:, 1:2, :], True)
        sm_min(i, 2)
        sm_min(i, 3)
        sm_exp(i, 2)
        sm_exp(i, 3)
        sm_fin(i, 1)
        sm_fin(i, 2)
        sm_fin(i, 3)

    def head_c(i, cb):
        # transpose the cb-sourced blocks of the unnormalized attention into
        # all 4 banks (cb-major: each batch runs right after exp cb, so only
        # the last batch waits for the final exp).  Reuses the energy banks
        # (tag "eb"); the fp8 cast-copies run on ACT (DVE owns the epilogue)
        p_sb = p_t[i]
        if cb == 0:
            del e_t[i]
            pt8_t[i] = pt_pool.tile([P, CB, 512], FP8, tag="pt", name=f"pt8_{i}")
            ptps_t[i] = ps_e.tile([P, CB * 512], FP32R, tag="eb", name=f"ptps{i}")
        pt_ps, pt8 = ptps_t[i], pt8_t[i]
        for db in range(CB):
            nc.tensor.matmul(
                pt_ps[:, db * 512 + cb * P : db * 512 + (cb + 1) * P],
                p_sb[:, cb * 512 + db * P : cb * 512 + (db + 1) * P],
                ident[:],
                is_transpose=True,
                start=(cb == 0),
                stop=(cb == CB - 1),
            )
        if cb == CB - 1:
            for db in range(CB):
                nc.scalar.copy(
                    pt8[:, db : db + 1, :], pt_ps[:, db * 512 : (db + 1) * 512]
                )
            del ptps_t[i]

    # per-slot schedule for next-sample work inside the 20-slot mm2 loop:
    # vt transposes paced to the v-load DMA arrival (2 chunks/slot), energy
    # chunks 2 behind, then the head (softmax / attention transpose) spread
    # over the late slots so its DVE/ACT latency hides under mm2+epilogue
    A_SCHED = {3: (0, 1, 2), 4: (3, 4, 5), 5: (6, 7), 6: (8, 9), 7: (10, 11),
               8: (12, 13), 9: (14, 15), 10: (16, 17)}
    E_SCHED = {5: (0, 1), 6: (2, 3), 7: (4, 5), 8: (6, 7), 9: (8, 9),
               10: (10, 11), 11: (12, 13), 12: (14, 15)}

    def emit(i):
        # Deeply software-pipelined: when emit(i) starts, sample i's vt,
        # energy, softmax and attention transpose are ALL already emitted
        # (the "head" ran interleaved into sample i-1's matmul phase).  This
        # emit runs sample i's mm2 + epilogue with sample i+1's work
        # interleaved per A_SCHED/E_SCHED.
        s = i % SPC
        v, vt, v8, pt8 = v_t[i], vt_t[i], v8_t.pop(i), pt8_t.pop(i)
        s_all = sa_t[i]
        nxt = i + 1 < nsamp
        if nxt:
            load_v(i + 1)

        slot = 0
        for cb in range(CB):
            ot = o_pool.tile([P, N], BF16, tag="ot")
            n_off = 0
            for nch in (512, 512, 512, 512, 256):
                po = ps_o.tile([P, 512], FP32, tag="po")
                for pair in range(2):
                    nc.tensor.matmul(
                        po[:, :nch],
                        pt8[:, 2 * pair : 2 * pair + 2, cb * P : (cb + 1) * P],
                        v8[:, 2 * pair : 2 * pair + 2, n_off : n_off + nch],
                        start=(pair == 0),
                        stop=(pair == 1),
                        perf_mode=DR,
                    )
                nc.vector.scalar_tensor_tensor(
                    ot[:, n_off : n_off + nch],
                    po[:, :nch],
                    s_all[:, cb : cb + 1],
                    v[:, cb * N + n_off : cb * N + n_off + nch].bitcast(FP32),
                    op0=OP.mult,
                    op1=OP.add,
                )
                n_off += nch
                slot += 1
                if nxt:
                    for k in A_SCHED.get(slot, ()):
                        a_chunk(i + 1, k)
                    for k in E_SCHED.get(slot, ()):
                        e_chunk(i + 1, k)
                    if slot == 9:
                        cast_v8(i + 1)
                    elif slot == 12:
                        head_a(i + 1)
                    elif slot == 13:
                        head_b(i + 1)
                    elif 14 <= slot <= 17:
                        head_c(i + 1, slot - 14)  # cb-major batch
            nc.sync.dma_start(out[s, cb * P : (cb + 1) * P, :], ot[:])
        del v_t[i], vt_t[i], p_t[i], sa_t[i]

    load_v(0)
    for k in range(KB):
        a_chunk(0, k)
    for k in range(KB - 2):
        e_chunk(0, k)
    cast_v8(0)
    head_a(0)
    head_b(0)
    for cb_ in range(CB):
        head_c(0, cb_)
    for i in range(nsamp):
        emit(i)


_nc_cache = {}


def _build(reps=1):
    if reps in _nc_cache:
        return _nc_cache[reps]
    nc = bacc.Bacc("TRN2", target_bir_lowering=False, debug=False)
    x_d = nc.dram_tensor("x", [SPC, C, N], FP32, kind="ExternalInput")
    g_d = nc.dram_tensor("gamma", [1], FP32, kind="ExternalInput")
    o_d = nc.dram_tensor("out", [SPC, C, N], BF16, kind="ExternalOutput")
    with tile.TileContext(nc) as tc, ExitStack() as ctx:
        _emit(tc, ctx, x_d.ap(), g_d.ap(), o_d.ap(), reps=reps)
    nc.compile()
    _nc_cache[reps] = nc
    return nc


def _bench_fn(reps, x, gamma):
    """Build a jitted 8-core executor for the reps-times-repeated kernel with
    device-resident inputs.  Used by test.py for differential timing."""
    import jax
    from jax.experimental.shard_map import shard_map
    from jax.sharding import Mesh, NamedSharding, PartitionSpec

    from concourse import bass2jax

    bass2jax.install_neuronx_cc_hook()
    nc = _build(reps=reps)
    pid = nc.partition_id_tensor.name if nc.partition_id_tensor else None
    in_names, out_names, out_avals, zero_outs = [], [], [], []
    for alloc in nc.m.functions[0].allocations:
        if not isinstance(alloc, mybir.MemoryLocationSet):
            continue
        name = alloc.memorylocations[0].name
        if alloc.kind == "ExternalInput":
            if name != pid:
                in_names.append(name)
        elif alloc.kind == "ExternalOutput":
            out_names.append(name)
            shape = tuple(alloc.tensor_shape)
            dtype = mybir.dt.np(alloc.dtype)
            out_avals.append(jax.core.ShapedArray(shape, dtype))
            zero_outs.append(np.zeros(shape, dtype))
    all_in_names = list(in_names) + list(out_names)
    if pid:
        all_in_names.append(pid)

    def _body(*args):
        operands = list(args)
        if pid:
            operands.append(bass2jax.partition_id_tensor())
        return tuple(
            bass2jax._bass_exec_p.bind(
                *operands,
                out_avals=tuple(out_avals),
                in_names=tuple(all_in_names),
                out_names=tuple(out_names),
                lowering_input_output_aliases=(),
                sim_require_finite=True,
                sim_require_nnan=True,
                nc=nc,
            )
        )

    devices = jax.devices()[:NCORES]
    mesh = Mesh(np.asarray(devices), ("core",))
    specs = (PartitionSpec("core"),) * (len(in_names) + len(out_names))
    fn = jax.jit(
        shard_map(
            _body,
            mesh=mesh,
            in_specs=specs,
            out_specs=(PartitionSpec("core"),) * len(out_names),
            check_rep=False,
        ),
        keep_unused=True,
    )
    sh = NamedSharding(mesh, PartitionSpec("core"))
    ins = {
        "x": np.ascontiguousarray(x, dtype=np.float32).reshape(B, C, N),
        "gamma": np.tile(np.ascontiguousarray(gamma, dtype=np.float32), (NCORES,)),
    }
    args = [jax.device_put(ins[n], sh) for n in in_names]
    args += [
        jax.device_put(np.zeros((NCORES * z.shape[0], *z.shape[1:]), z.dtype), sh)
        for z in zero_outs
    ]
    return fn, args


def kernel(x: np.ndarray, gamma: np.ndarray, **run_kwargs) -> np.ndarray:
    assert x.shape == (B, C, H, W), x.shape
    nc = _build()
    xr = np.ascontiguousarray(x, dtype=np.float32).reshape(B, C, N)
    g = np.ascontiguousarray(gamma, dtype=np.float32)
    in_maps = [
        {"x": xr[g_idx * SPC : (g_idx + 1) * SPC], "gamma": g}
        for g_idx in range(NCORES)
    ]
    res = run_bass_kernel_spmd(nc, in_maps, core_ids=list(range(NCORES)), **run_kwargs)
    outs = [np.asarray(res.results[g_idx]["out"]) for g_idx in range(NCORES)]
    full = (
        np.concatenate(outs, axis=0)
        .astype(np.float32)
        .reshape(B, C, H, W)
    )
    if run_kwargs:
        kernel.last_results = res
    return full
